# revision 10
# baseline (speedup 1.0000x reference)
"""2-layer GAT (PyG GATConv semantics) forward on 8 Trainium2 NeuronCores.

Strategy (edge/destination-parallel, per spec sharding hint):
  - Self-loops appended, edges sorted by destination on the host.
  - Nodes padded to NPAD = 8*B*128 and split into 8 contiguous ranges
    (one per core); each core owns all edges whose dst is in its range.
  - Edges of each 128-node block are padded to T tiles of 128 edges
    (sentinel edges have dst_local = -1 so they vanish from the
    aggregation matmul), giving a fully static, SPMD-uniform program:
    per-core differences live entirely in the input data.
  - Phase 1 (replicated): h_cat = x @ W1_cat on every core, where W1_cat
    packs, per head, [32 feature cols | a_src col] (33 cols/head) plus 8
    a_dst cols.  Attention projections are folded into the weight matrix
    on the host (W1 @ blockdiag(att)), so one matmul produces features
    and attention logit halves.
  - Phase 2: per 128-node block, one multi-row indirect DMA gathers all
    T*128 source rows; logits = leaky_relu(a_src[src]+a_dst[dst]);
    w = exp(logits) (softmax max-shift dropped: logits are O(1));
    messages = rows * w; a [128e x 128n] 0/1 matrix M (dst_local == j)
    maps edge tiles onto block nodes via PE matmuls accumulated in PSUM.
    The a_src column is overwritten with 1.0 before weighting so the
    same matmul also accumulates the softmax denominators.  Epilogue:
    divide, +bias, ELU, PE-transpose into an h1T [256 x npc] table.
  - Phase 3: h2_cat = h1 @ W2_cat (40 cols + a_src2 + a_dst2).
  - Phase 4: AllGather of the small h2_cat tables (8 cores).
  - Phase 5: same edge machinery for layer 2 (1 head), log_softmax
    epilogue, per-core output rows.
"""

import os
import sys

import numpy as np

for _p in ("/opt/trn_rl_repo", "/root/.axon_site/_ro/trn_rl_repo"):
    if os.path.isdir(_p) and _p not in sys.path:
        sys.path.insert(0, _p)

P = 128
NCORES = 8
IN_F, HID, HEADS, NCLASS = 256, 32, 8, 40
NEG_SLOPE = 0.2
CH = HID + 1          # 33 cols per head in h_cat (32 features + a_src)
HC = HEADS * CH       # 264
HD = HEADS * HID      # 256
C2 = NCLASS + 2       # 42 = 40 features + a_src2 + a_dst2
EPS = 1e-9


def _prep_graph(src, dst, nblocks):
    """Sort edges by dst, split into 128-node blocks, pad each block to a
    uniform T tiles of 128 edges.  Returns tile-layout index arrays
    [nblocks, 128, T]: gather index (src node), dst_local (f32, -1 pad),
    dst global (int32)."""
    order = np.argsort(dst, kind="stable")
    src_s = src[order].astype(np.int32)
    dst_s = dst[order].astype(np.int32)
    blk = dst_s // P
    cnt = np.bincount(blk, minlength=nblocks)
    T = max(1, int(-(-int(cnt.max()) // P)))
    gidx = np.zeros((nblocks, P, T), np.int32)
    dstl = np.full((nblocks, P, T), -1.0, np.float32)
    dstg = np.zeros((nblocks, P, T), np.int32)
    starts = np.concatenate([[0], np.cumsum(cnt)]).astype(np.int64)
    for b in range(nblocks):
        s, e = int(starts[b]), int(starts[b + 1])
        L = e - s
        bs = np.zeros(T * P, np.int32)
        bl = np.full(T * P, -1.0, np.float32)
        bg = np.zeros(T * P, np.int32)
        bs[:L] = src_s[s:e]
        bl[:L] = (dst_s[s:e] - b * P).astype(np.float32)
        bg[:L] = dst_s[s:e]
        gidx[b] = bs.reshape(T, P).T
        dstl[b] = bl.reshape(T, P).T
        dstg[b] = bg.reshape(T, P).T
    return gidx, dstl, dstg, T


def _fix_dma_waits(nc, max_waits=2):
    """Walrus's codegen supports at most two semaphore wait commands per
    instruction.  For any instruction the Tile scheduler gave more waits,
    hoist the excess onto a chain of nops on the same engine, each carrying
    at most two waits — the sequencer issues in order, so the instruction
    still starts only after all waits are satisfied."""
    import concourse.mybir as mybir

    n = 0
    for bb in nc.m.functions[0].blocks:
        out = []
        for ins in bb.instructions:
            si = ins.sync_info
            if si is not None and len(si.on_wait) > max_waits and ins.is_executable():
                waits = list(si.on_wait)
                keep = waits[:max_waits]
                rest = waits[max_waits:]
                while rest:
                    chunk, rest = rest[:max_waits], rest[max_waits:]
                    nop = mybir.InstNoOp(name=f"I-waitfix-{n}", ins=[], outs=[])
                    n += 1
                    nop.engine = ins.engine
                    nop.sync_info = mybir.SyncInfo(on_wait=chunk, on_update=[])
                    out.append(nop)
                ins.sync_info = mybir.SyncInfo(
                    on_wait=keep, on_update=list(si.on_update)
                )
            out.append(ins)
        bb.instructions = out
    return n


def _build_program(npad, npc, B, T, fix_waits=True):
    import concourse.bacc as bacc
    import concourse.bass as bass
    import concourse.mybir as mybir
    import concourse.tile as tile

    f32 = mybir.dt.float32
    i32 = mybir.dt.int32
    AF = mybir.ActivationFunctionType
    OP = mybir.AluOpType
    X = mybir.AxisListType.X

    nc = bacc.Bacc("TRN2")

    xT = nc.dram_tensor("xT", [IN_F, npad], f32, kind="ExternalInput")
    w1c = nc.dram_tensor("w1c", [IN_F, HC + HEADS], f32, kind="ExternalInput")
    w2c = nc.dram_tensor("w2c", [HD, C2], f32, kind="ExternalInput")
    gidx = nc.dram_tensor("gidx", [B, P, T], i32, kind="ExternalInput")
    dstl = nc.dram_tensor("dstl", [B, P, T], f32, kind="ExternalInput")
    dstg = nc.dram_tensor("dstg", [B, P, T], i32, kind="ExternalInput")
    b1r = nc.dram_tensor("b1r", [P, HD], f32, kind="ExternalInput")
    b2r = nc.dram_tensor("b2r", [P, NCLASS], f32, kind="ExternalInput")
    c128 = nc.dram_tensor("c128", [P, P], f32, kind="ExternalInput")
    idn = nc.dram_tensor("idn", [P, P], f32, kind="ExternalInput")

    hcat = nc.dram_tensor("hcat", [npad, HC], f32)
    adt1 = nc.dram_tensor("adt1", [npad, HEADS], f32)
    h1T = nc.dram_tensor("h1T", [HD, npc], f32)
    h2loc = nc.dram_tensor("h2loc", [npc, C2], f32)
    h2tab = nc.dram_tensor("h2tab", [npad, C2], f32, addr_space="Shared")
    outx = nc.dram_tensor("out", [npc, NCLASS], f32, kind="ExternalOutput")

    with tile.TileContext(nc) as tc:
        with tc.tile_pool(name="const", bufs=1) as cp:
            w1a = cp.tile([P, HC + HEADS], f32)
            nc.sync.dma_start(w1a[:], w1c[0:P, :])
            w1b = cp.tile([P, HC + HEADS], f32)
            nc.sync.dma_start(w1b[:], w1c[P : 2 * P, :])
            w2a = cp.tile([P, C2], f32)
            nc.sync.dma_start(w2a[:], w2c[0:P, :])
            w2b = cp.tile([P, C2], f32)
            nc.sync.dma_start(w2b[:], w2c[P : 2 * P, :])
            b1t = cp.tile([P, HD], f32)
            nc.sync.dma_start(b1t[:], b1r[:])
            b2t = cp.tile([P, NCLASS], f32)
            nc.sync.dma_start(b2t[:], b2r[:])
            ct = cp.tile([P, P], f32)
            nc.sync.dma_start(ct[:], c128[:])
            idt = cp.tile([P, P], f32)
            nc.sync.dma_start(idt[:], idn[:])

            # ---- Phase 1: h_cat = x @ W1_cat (replicated on every core)
            with tc.tile_pool(name="p1", bufs=3) as p1, tc.tile_pool(
                name="p1ps", bufs=2, space="PSUM"
            ) as p1ps:
                for nt in range(npad // P):
                    sl = slice(nt * P, (nt + 1) * P)
                    xa = p1.tile([P, P], f32, tag="xa")
                    nc.sync.dma_start(xa[:], xT[0:P, sl])
                    xb = p1.tile([P, P], f32, tag="xb")
                    nc.sync.dma_start(xb[:], xT[P : 2 * P, sl])
                    ps = p1ps.tile([P, HC + HEADS], f32, tag="p1ps")
                    nc.tensor.matmul(ps[:], xa[:], w1a[:], start=True, stop=False)
                    nc.tensor.matmul(ps[:], xb[:], w1b[:], start=False, stop=True)
                    hb = p1.tile([P, HC + HEADS], f32, tag="hb")
                    nc.vector.tensor_copy(hb[:], ps[:])
                    nc.sync.dma_start(hcat[sl, :], hb[:, 0:HC])
                    nc.sync.dma_start(adt1[sl, :], hb[:, HC : HC + HEADS])
            tc.strict_bb_all_engine_barrier()

            # ---- Phase 2: layer-1 edge aggregation, one 128-node block at a time
            with tc.tile_pool(name="p2", bufs=2) as p2, tc.tile_pool(
                name="p2ps", bufs=2, space="PSUM"
            ) as p2ps:
                for b in range(B):
                    nsl = slice(b * P, (b + 1) * P)
                    it = p2.tile([P, T], i32, tag="it")
                    nc.sync.dma_start(it[:], gidx[b])
                    dl = p2.tile([P, T], f32, tag="dl")
                    nc.sync.dma_start(dl[:], dstl[b])
                    dg = p2.tile([P, T], i32, tag="dg")
                    nc.sync.dma_start(dg[:], dstg[b])
                    hs = p2.tile([P, T * HC], f32, tag="hs")
                    nc.gpsimd.indirect_dma_start(
                        out=hs[:],
                        out_offset=None,
                        in_=hcat[:, :],
                        in_offset=bass.IndirectOffsetOnAxis(ap=it[:, :], axis=0),
                    )
                    ad = p2.tile([P, T * HEADS], f32, tag="ad")
                    nc.gpsimd.indirect_dma_start(
                        out=ad[:],
                        out_offset=None,
                        in_=adt1[:, :],
                        in_offset=bass.IndirectOffsetOnAxis(ap=dg[:, :], axis=0),
                    )
                    hs4 = hs[:].rearrange("p (t h c) -> p t h c", h=HEADS, c=CH)
                    # logits = a_src[src] + a_dst[dst]; leaky_relu; exp
                    lg = p2.tile([P, T * HEADS], f32, tag="lg")
                    nc.vector.tensor_tensor(
                        out=lg[:].rearrange("p (t h) -> p t h", h=HEADS),
                        in0=hs4[:, :, :, HID],
                        in1=ad[:].rearrange("p (t h) -> p t h", h=HEADS),
                        op=OP.add,
                    )
                    lr = p2.tile([P, T * HEADS], f32, tag="lr")
                    nc.vector.tensor_scalar_mul(lr[:], lg[:], NEG_SLOPE)
                    nc.vector.tensor_tensor(out=lr[:], in0=lr[:], in1=lg[:], op=OP.max)
                    wv = p2.tile([P, T * HEADS], f32, tag="wv")
                    nc.scalar.activation(wv[:], lr[:], AF.Exp)
                    # denominator trick: a_src slot := 1.0 so the same matmul
                    # accumulates sum(w) per (node, head)
                    nc.vector.memset(hs4[:, :, :, HID : HID + 1], 1.0)
                    mm = p2.tile([P, T * P], f32, tag="mm")
                    nc.vector.tensor_tensor(
                        out=mm[:].rearrange("p (t j) -> p t j", j=P),
                        in0=dl[:, :, None].to_broadcast([P, T, P]),
                        in1=ct[:, None, :].to_broadcast([P, T, P]),
                        op=OP.is_equal,
                    )
                    ms = p2.tile([P, T * HC], f32, tag="ms")
                    nc.vector.tensor_tensor(
                        out=ms[:].rearrange("p (t h c) -> p t h c", h=HEADS, c=CH),
                        in0=hs4,
                        in1=wv[:]
                        .rearrange("p (t h) -> p t h", h=HEADS)[:, :, :, None]
                        .to_broadcast([P, T, HEADS, CH]),
                        op=OP.mult,
                    )
                    ag = p2ps.tile([P, HC], f32, tag="ag")
                    for t in range(T):
                        nc.tensor.matmul(
                            ag[:],
                            mm[:, t * P : (t + 1) * P],
                            ms[:, t * HC : (t + 1) * HC],
                            start=(t == 0),
                            stop=(t == T - 1),
                        )
                    ag3 = ag[:].rearrange("p (h c) -> p h c", c=CH)
                    dpre = p2.tile([P, HEADS], f32, tag="dpre")
                    nc.vector.tensor_scalar_add(
                        dpre[:].rearrange("p (h o) -> p h o", o=1),
                        ag3[:, :, HID : HID + 1],
                        EPS,
                    )
                    dr = p2.tile([P, HEADS], f32, tag="dr")
                    nc.vector.reciprocal(dr[:], dpre[:])
                    h1 = p2.tile([P, HD], f32, tag="h1")
                    nc.vector.tensor_tensor(
                        out=h1[:].rearrange("p (h c) -> p h c", c=HID),
                        in0=ag3[:, :, 0:HID],
                        in1=dr[:, :, None].to_broadcast([P, HEADS, HID]),
                        op=OP.mult,
                    )
                    nc.vector.tensor_add(h1[:], h1[:], b1t[:])
                    # ELU(x) = relu(x) + exp(min(x,0)) - 1
                    tmn = p2.tile([P, HD], f32, tag="tmn")
                    nc.vector.tensor_scalar_min(tmn[:], h1[:], 0.0)
                    tex = p2.tile([P, HD], f32, tag="tex")
                    nc.scalar.activation(tex[:], tmn[:], AF.Exp)
                    trl = p2.tile([P, HD], f32, tag="trl")
                    nc.scalar.activation(trl[:], h1[:], AF.Relu)
                    nc.vector.tensor_add(tex[:], tex[:], trl[:])
                    nc.vector.tensor_scalar_add(tex[:], tex[:], -1.0)
                    for half in range(2):
                        tp = p2ps.tile([P, P], f32, tag="tp")
                        nc.tensor.transpose(
                            tp[:], tex[:, half * P : (half + 1) * P], idt[:]
                        )
                        ccp = p2.tile([P, P], f32, tag="ccp")
                        nc.vector.tensor_copy(ccp[:], tp[:])
                        nc.sync.dma_start(h1T[half * P : (half + 1) * P, nsl], ccp[:])
            tc.strict_bb_all_engine_barrier()

            # ---- Phase 3: h2_cat = h1 @ W2_cat for this core's nodes
            with tc.tile_pool(name="p3", bufs=3) as p3, tc.tile_pool(
                name="p3ps", bufs=2, space="PSUM"
            ) as p3ps:
                for ntl in range(npc // P):
                    sl = slice(ntl * P, (ntl + 1) * P)
                    ha = p3.tile([P, P], f32, tag="ha")
                    nc.sync.dma_start(ha[:], h1T[0:P, sl])
                    hb2 = p3.tile([P, P], f32, tag="hb2")
                    nc.sync.dma_start(hb2[:], h1T[P : 2 * P, sl])
                    ps3 = p3ps.tile([P, C2], f32, tag="p3ps")
                    nc.tensor.matmul(ps3[:], ha[:], w2a[:], start=True, stop=False)
                    nc.tensor.matmul(ps3[:], hb2[:], w2b[:], start=False, stop=True)
                    hc2 = p3.tile([P, C2], f32, tag="hc2")
                    nc.vector.tensor_copy(hc2[:], ps3[:])
                    nc.sync.dma_start(h2loc[sl, :], hc2[:])
            tc.strict_bb_all_engine_barrier()

            # ---- Phase 4: AllGather the per-core h2_cat slices
            nc.gpsimd.collective_compute(
                "AllGather",
                OP.bypass,
                replica_groups=[list(range(NCORES))],
                ins=[h2loc[:]],
                outs=[h2tab[:]],
            )
            tc.strict_bb_all_engine_barrier()

            # ---- Phase 5: layer-2 edge aggregation + log_softmax
            with tc.tile_pool(name="p5", bufs=2) as p5, tc.tile_pool(
                name="p5ps", bufs=2, space="PSUM"
            ) as p5ps:
                for b in range(B):
                    nsl = slice(b * P, (b + 1) * P)
                    it = p5.tile([P, T], i32, tag="it5")
                    nc.sync.dma_start(it[:], gidx[b])
                    dl = p5.tile([P, T], f32, tag="dl5")
                    nc.sync.dma_start(dl[:], dstl[b])
                    dg = p5.tile([P, T], i32, tag="dg5")
                    nc.sync.dma_start(dg[:], dstg[b])
                    hs = p5.tile([P, T * C2], f32, tag="hs5")
                    nc.gpsimd.indirect_dma_start(
                        out=hs[:],
                        out_offset=None,
                        in_=h2tab[:, :],
                        in_offset=bass.IndirectOffsetOnAxis(ap=it[:, :], axis=0),
                    )
                    ad2 = p5.tile([P, T], f32, tag="ad5")
                    nc.gpsimd.indirect_dma_start(
                        out=ad2[:],
                        out_offset=None,
                        in_=h2tab[:, :],
                        in_offset=bass.IndirectOffsetOnAxis(ap=dg[:, :], axis=0),
                        element_offset=NCLASS + 1,
                    )
                    hs3 = hs[:].rearrange("p (t c) -> p t c", c=C2)
                    lg = p5.tile([P, T], f32, tag="lg5")
                    nc.vector.tensor_tensor(
                        out=lg[:].rearrange("p (t o) -> p t o", o=1),
                        in0=hs3[:, :, NCLASS : NCLASS + 1],
                        in1=ad2[:, :, None],
                        op=OP.add,
                    )
                    lr = p5.tile([P, T], f32, tag="lr5")
                    nc.vector.tensor_scalar_mul(lr[:], lg[:], NEG_SLOPE)
                    nc.vector.tensor_tensor(out=lr[:], in0=lr[:], in1=lg[:], op=OP.max)
                    wv = p5.tile([P, T], f32, tag="wv5")
                    nc.scalar.activation(wv[:], lr[:], AF.Exp)
                    nc.vector.memset(hs3[:, :, NCLASS : NCLASS + 1], 1.0)
                    mm = p5.tile([P, T * P], f32, tag="mm5")
                    nc.vector.tensor_tensor(
                        out=mm[:].rearrange("p (t j) -> p t j", j=P),
                        in0=dl[:, :, None].to_broadcast([P, T, P]),
                        in1=ct[:, None, :].to_broadcast([P, T, P]),
                        op=OP.is_equal,
                    )
                    ms = p5.tile([P, T * C2], f32, tag="ms5")
                    nc.vector.tensor_tensor(
                        out=ms[:].rearrange("p (t c) -> p t c", c=C2),
                        in0=hs3,
                        in1=wv[:, :, None].to_broadcast([P, T, C2]),
                        op=OP.mult,
                    )
                    ag = p5ps.tile([P, C2], f32, tag="ag5")
                    for t in range(T):
                        nc.tensor.matmul(
                            ag[:],
                            mm[:, t * P : (t + 1) * P],
                            ms[:, t * C2 : (t + 1) * C2],
                            start=(t == 0),
                            stop=(t == T - 1),
                        )
                    dpre = p5.tile([P, 1], f32, tag="dpre5")
                    nc.vector.tensor_scalar_add(
                        dpre[:], ag[:, NCLASS : NCLASS + 1], EPS
                    )
                    dr = p5.tile([P, 1], f32, tag="dr5")
                    nc.vector.reciprocal(dr[:], dpre[:])
                    o2 = p5.tile([P, NCLASS], f32, tag="o2")
                    nc.vector.tensor_tensor(
                        out=o2[:],
                        in0=ag[:, 0:NCLASS],
                        in1=dr[:].to_broadcast([P, NCLASS]),
                        op=OP.mult,
                    )
                    nc.vector.tensor_add(o2[:], o2[:], b2t[:])
                    mx = p5.tile([P, 1], f32, tag="mx")
                    nc.vector.tensor_reduce(mx[:], o2[:], axis=X, op=OP.max)
                    nmx = p5.tile([P, 1], f32, tag="nmx")
                    nc.vector.tensor_scalar_mul(nmx[:], mx[:], -1.0)
                    ex = p5.tile([P, NCLASS], f32, tag="ex")
                    sm = p5.tile([P, 1], f32, tag="sm")
                    nc.scalar.activation(
                        ex[:], o2[:], AF.Exp, bias=nmx[:, 0:1], accum_out=sm[:]
                    )
                    ls = p5.tile([P, 1], f32, tag="ls")
                    nc.scalar.activation(ls[:], sm[:], AF.Ln)
                    sh = p5.tile([P, 1], f32, tag="sh")
                    nc.vector.tensor_tensor(
                        out=sh[:], in0=nmx[:], in1=ls[:], op=OP.subtract
                    )
                    fo = p5.tile([P, NCLASS], f32, tag="fo")
                    nc.scalar.activation(fo[:], o2[:], AF.Identity, bias=sh[:, 0:1])
                    nc.sync.dma_start(outx[nsl, :], fo[:])
    nc.compile()
    return nc


def _host_prep(inputs):
    x = np.ascontiguousarray(np.asarray(inputs["x"], np.float32))
    ei = np.asarray(inputs["edge_index"]).astype(np.int64)
    W1 = np.asarray(inputs["W1"], np.float32)
    as1 = np.asarray(inputs["att_src1"], np.float32)
    ad1 = np.asarray(inputs["att_dst1"], np.float32)
    b1 = np.asarray(inputs["b1"], np.float32)
    W2 = np.asarray(inputs["W2"], np.float32)
    as2 = np.asarray(inputs["att_src2"], np.float32).reshape(-1)
    ad2 = np.asarray(inputs["att_dst2"], np.float32).reshape(-1)
    b2 = np.asarray(inputs["b2"], np.float32)

    n = x.shape[0]
    B = -(-n // (NCORES * P))  # blocks per core
    npc = B * P
    npad = NCORES * npc

    loops = np.arange(n, dtype=np.int64)
    src = np.concatenate([ei[0], loops])
    dst = np.concatenate([ei[1], loops])
    gidx, dstl, dstg, T = _prep_graph(src, dst, NCORES * B)

    w1cat = np.zeros((IN_F, HC + HEADS), np.float32)
    for h in range(HEADS):
        w1cat[:, h * CH : h * CH + HID] = W1[:, h * HID : (h + 1) * HID]
        w1cat[:, h * CH + HID] = W1[:, h * HID : (h + 1) * HID] @ as1[h]
        w1cat[:, HC + h] = W1[:, h * HID : (h + 1) * HID] @ ad1[h]
    w2cat = np.zeros((HD, C2), np.float32)
    w2cat[:, :NCLASS] = W2
    w2cat[:, NCLASS] = W2 @ as2
    w2cat[:, NCLASS + 1] = W2 @ ad2

    xT = np.zeros((IN_F, npad), np.float32)
    xT[:, :n] = x.T

    host = dict(
        xT=xT,
        w1c=w1cat,
        w2c=w2cat,
        b1r=np.tile(b1[None, :], (P, 1)).astype(np.float32),
        b2r=np.tile(b2[None, :], (P, 1)).astype(np.float32),
        c128=np.tile(np.arange(P, dtype=np.float32)[None, :], (P, 1)),
        idn=np.eye(P, dtype=np.float32),
    )
    in_maps = []
    for c in range(NCORES):
        m = dict(host)
        m["gidx"] = np.ascontiguousarray(gidx[c * B : (c + 1) * B])
        m["dstl"] = np.ascontiguousarray(dstl[c * B : (c + 1) * B])
        m["dstg"] = np.ascontiguousarray(dstg[c * B : (c + 1) * B])
        in_maps.append(m)
    return in_maps, n, npad, npc, B, T


def _forward(inputs, trace=False):
    from concourse.bass_utils import run_bass_kernel_spmd

    in_maps, n, npad, npc, B, T = _host_prep(inputs)
    nc = _build_program(npad, npc, B, T)
    res = run_bass_kernel_spmd(nc, in_maps, list(range(NCORES)), trace=trace)
    out = np.concatenate([res.results[c]["out"] for c in range(NCORES)], axis=0)
    return out[:n].astype(np.float32), res


def kernel(**inputs) -> np.ndarray:
    out, _ = _forward(inputs, trace=False)
    return out


# revision 11
# speedup vs baseline: 1.8778x; 1.8778x over previous
"""2-layer GAT (PyG GATConv semantics) forward on 8 Trainium2 NeuronCores.

Strategy (edge/destination-parallel, per spec sharding hint):
  - Self-loops appended, edges sorted by destination on the host.
  - Nodes padded to NPAD = 8*B*128 and split into 8 contiguous ranges
    (one per core); each core owns all edges whose dst is in its range.
  - Edges of each 128-node block are padded to T tiles of 128 edges
    (sentinel edges have dst_local = -1 so they vanish from the
    aggregation matmul), giving a fully static, SPMD-uniform program:
    per-core differences live entirely in the input data.
  - Phase 1 (replicated): hcat = x @ W1_cat on every core, where W1_cat
    packs, per head, [32 feature cols | a_src col] (33 cols/head) plus 8
    a_dst cols.  Attention projections are folded into the weight matrix
    on the host (W1 @ blockdiag(att)), so one matmul produces features
    and attention logit halves.  The table is stored in bf16.
  - Phase 2: per 128-node block, one multi-row indirect DMA gathers all
    T*128 source rows; logits = leaky_relu(a_src[src]+a_dst[dst]);
    w = exp(logits) (softmax max-shift dropped: logits are O(1));
    messages = rows * w; a [128e x 128n] 0/1 matrix M (dst_local == j)
    maps edge tiles onto block nodes via PE matmuls accumulated in PSUM.
    The a_src column is overwritten with 1.0 before weighting so the
    same matmul also accumulates the softmax denominators.  Epilogue:
    divide, +bias, ELU, PE-transpose into an h1T [256 x npc] table.
  - Phase 3: h2_cat = h1 @ W2_cat (40 cols + a_src2 + a_dst2).
  - Phase 4: AllGather of the small h2_cat tables (8 cores).
  - Phase 5: same edge machinery for layer 2 (1 head); the log_softmax
    ln() is deferred and batched over all blocks to avoid ACT
    activation-table reloads.
"""

import os
import sys

import numpy as np

for _p in ("/opt/trn_rl_repo", "/root/.axon_site/_ro/trn_rl_repo"):
    if os.path.isdir(_p) and _p not in sys.path:
        sys.path.insert(0, _p)

P = 128
NCORES = 8
IN_F, HID, HEADS, NCLASS = 256, 32, 8, 40
NEG_SLOPE = 0.2
CH = HID + 1          # 33 cols per head (32 features + a_src)
HC = HEADS * CH       # 264
HCX = HC + HEADS      # 272 = hcat row incl the 8 a_dst cols
HD = HEADS * HID      # 256
C2 = NCLASS + 2       # 42 = 40 features + a_src2 + a_dst2
EPS = 1e-9


def _prep_graph(src, dst, nblocks):
    """Sort edges by dst, split into 128-node blocks, pad each block to a
    uniform T tiles of 128 edges.  Returns tile-layout index arrays
    [nblocks, 128, T]: gather index (src node), dst global (int32),
    dst_local (f32, -1 pad)."""
    order = np.argsort(dst, kind="stable")
    src_s = src[order].astype(np.int32)
    dst_s = dst[order].astype(np.int32)
    blk = dst_s // P
    cnt = np.bincount(blk, minlength=nblocks)
    T = max(1, int(-(-int(cnt.max()) // P)))
    gidx = np.zeros((nblocks, P, T), np.int32)
    dstg = np.zeros((nblocks, P, T), np.int32)
    dstl = np.full((nblocks, P, T), -1.0, np.float32)
    starts = np.concatenate([[0], np.cumsum(cnt)]).astype(np.int64)
    for b in range(nblocks):
        s, e = int(starts[b]), int(starts[b + 1])
        L = e - s
        bs = np.zeros(T * P, np.int32)
        bg = np.zeros(T * P, np.int32)
        bl = np.full(T * P, -1.0, np.float32)
        bs[:L] = src_s[s:e]
        bg[:L] = dst_s[s:e]
        bl[:L] = (dst_s[s:e] - b * P).astype(np.float32)
        gidx[b] = bs.reshape(T, P).T
        dstg[b] = bg.reshape(T, P).T
        dstl[b] = bl.reshape(T, P).T
    return gidx, dstg, dstl, T


def _build_program(npad, npc, B, T):
    import concourse.bacc as bacc
    import concourse.bass as bass
    import concourse.mybir as mybir
    import concourse.tile as tile

    f32 = mybir.dt.float32
    bf16 = mybir.dt.bfloat16
    i32 = mybir.dt.int32
    AF = mybir.ActivationFunctionType
    OP = mybir.AluOpType
    X = mybir.AxisListType.X

    nc = bacc.Bacc("TRN2")

    NT4 = npad // (4 * P)  # phase-1 iterations, 4 row-tiles each

    xT = nc.dram_tensor("xT", [IN_F, npad], bf16, kind="ExternalInput")
    w1c = nc.dram_tensor("w1c", [IN_F, HCX], bf16, kind="ExternalInput")
    w2c = nc.dram_tensor("w2c", [HD, C2], bf16, kind="ExternalInput")
    # per-block metadata, packed: [gidx | dstg | dstl(f32 bits)]
    meta = nc.dram_tensor("meta", [B, P, 3 * T], i32, kind="ExternalInput")
    b1r = nc.dram_tensor("b1r", [P, HD], f32, kind="ExternalInput")
    b2r = nc.dram_tensor("b2r", [P, NCLASS], f32, kind="ExternalInput")
    c128 = nc.dram_tensor("c128", [P, P], bf16, kind="ExternalInput")
    idn = nc.dram_tensor("idn", [P, P], f32, kind="ExternalInput")

    hcat = nc.dram_tensor("hcat", [npad, HCX], bf16)
    h1T = nc.dram_tensor("h1T", [HD, npc], bf16)
    h2loc = nc.dram_tensor("h2loc", [npc, C2], bf16)
    h2tab = nc.dram_tensor("h2tab", [npad, C2], bf16, addr_space="Shared")
    outx = nc.dram_tensor("out", [npc, NCLASS], f32, kind="ExternalOutput")

    with tile.TileContext(nc) as tc:
        with tc.tile_pool(name="const", bufs=1) as cp:
            w1a = cp.tile([P, HCX], bf16)
            nc.sync.dma_start(w1a[:], w1c[0:P, :])
            w1b = cp.tile([P, HCX], bf16)
            nc.sync.dma_start(w1b[:], w1c[P : 2 * P, :])
            w2a = cp.tile([P, C2], bf16)
            nc.sync.dma_start(w2a[:], w2c[0:P, :])
            w2b = cp.tile([P, C2], bf16)
            nc.sync.dma_start(w2b[:], w2c[P : 2 * P, :])
            b1t = cp.tile([P, HD], f32)
            nc.sync.dma_start(b1t[:], b1r[:])
            b2t = cp.tile([P, NCLASS], f32)
            nc.sync.dma_start(b2t[:], b2r[:])
            ct = cp.tile([P, P], bf16)
            nc.sync.dma_start(ct[:], c128[:])
            idt = cp.tile([P, P], f32)
            nc.sync.dma_start(idt[:], idn[:])
            # log_softmax cross-block accumulators (one col per block)
            smx = cp.tile([P, B], f32)   # sum(exp(o2 - max))
            nmx = cp.tile([P, B], f32)   # -max
            o2all = cp.tile([P, B * NCLASS], f32)

            # ---- Phase 1: hcat = x @ W1_cat (replicated on every core)
            with tc.tile_pool(name="p1", bufs=3) as p1, tc.tile_pool(
                name="p1ps", bufs=8, space="PSUM"
            ) as p1ps:
                for nt in range(NT4):
                    sl4 = slice(nt * 4 * P, (nt + 1) * 4 * P)
                    xa = p1.tile([P, 4 * P], bf16, tag="xa")
                    nc.scalar.dma_start(xa[:], xT[0:P, sl4])
                    xb = p1.tile([P, 4 * P], bf16, tag="xb")
                    nc.scalar.dma_start(xb[:], xT[P : 2 * P, sl4])
                    hb = p1.tile([P, 4 * HCX], bf16, tag="hb")
                    for q in range(4):
                        ps = p1ps.tile([P, HCX], f32, tag="p1ps")
                        nc.tensor.matmul(
                            ps[:],
                            xa[:, q * P : (q + 1) * P],
                            w1a[:],
                            start=True,
                            stop=False,
                        )
                        nc.tensor.matmul(
                            ps[:],
                            xb[:, q * P : (q + 1) * P],
                            w1b[:],
                            start=False,
                            stop=True,
                        )
                        nc.vector.tensor_copy(
                            hb[:, q * HCX : (q + 1) * HCX], ps[:]
                        )
                    nc.sync.dma_start(
                        hcat[sl4, :].rearrange("(a p) c -> p a c", p=P), hb[:]
                    )
            tc.strict_bb_all_engine_barrier()

            # ---- Phase 2: layer-1 edge aggregation, one 128-node block at a time
            with tc.tile_pool(name="p2", bufs=2) as p2, tc.tile_pool(
                name="p2ps", bufs=2, space="PSUM"
            ) as p2ps:
                for b in range(B):
                    nsl = slice(b * P, (b + 1) * P)
                    mt = p2.tile([P, 3 * T], i32, tag="mt")
                    nc.scalar.dma_start(mt[:], meta[b])
                    it = mt[:, 0:T]
                    dg = mt[:, T : 2 * T]
                    dl = mt[:, 2 * T : 3 * T].bitcast(f32)
                    dlb = p2.tile([P, T], bf16, tag="dlb")
                    nc.vector.tensor_copy(dlb[:], dl)
                    hs = p2.tile([P, T * HCX], bf16, tag="hs")
                    nc.gpsimd.indirect_dma_start(
                        out=hs[:],
                        out_offset=None,
                        in_=hcat[:, :],
                        in_offset=bass.IndirectOffsetOnAxis(ap=it, axis=0),
                    )
                    ad = p2.tile([P, T * HEADS], bf16, tag="ad")
                    nc.gpsimd.indirect_dma_start(
                        out=ad[:],
                        out_offset=None,
                        in_=hcat[:, :],
                        in_offset=bass.IndirectOffsetOnAxis(ap=dg, axis=0),
                        element_offset=HC,
                    )
                    hs4 = hs[:].rearrange("p (t x) -> p t x", x=HCX)[
                        :, :, 0:HC
                    ].rearrange("p t (h c) -> p t h c", c=CH)
                    # logits = a_src[src] + a_dst[dst]; leaky_relu; exp
                    lg = p2.tile([P, T * HEADS], f32, tag="lg")
                    nc.vector.tensor_tensor(
                        out=lg[:].rearrange("p (t h) -> p t h", h=HEADS),
                        in0=hs4[:, :, :, HID],
                        in1=ad[:].rearrange("p (t h) -> p t h", h=HEADS),
                        op=OP.add,
                    )
                    lr = p2.tile([P, T * HEADS], f32, tag="lr")
                    nc.vector.tensor_scalar_mul(lr[:], lg[:], NEG_SLOPE)
                    nc.vector.tensor_tensor(out=lr[:], in0=lr[:], in1=lg[:], op=OP.max)
                    wv = p2.tile([P, T * HEADS], bf16, tag="wv")
                    nc.scalar.activation(wv[:], lr[:], AF.Exp)
                    # denominator trick: a_src slot := 1.0 so the same matmul
                    # accumulates sum(w) per (node, head)
                    nc.vector.memset(hs4[:, :, :, HID : HID + 1], 1.0)
                    mm = p2.tile([P, T * P], bf16, tag="mm")
                    nc.vector.tensor_tensor(
                        out=mm[:].rearrange("p (t j) -> p t j", j=P),
                        in0=dlb[:, :, None].to_broadcast([P, T, P]),
                        in1=ct[:, None, :].to_broadcast([P, T, P]),
                        op=OP.is_equal,
                    )
                    ms = p2.tile([P, T * HC], bf16, tag="ms")
                    nc.vector.tensor_tensor(
                        out=ms[:].rearrange("p (t h c) -> p t h c", h=HEADS, c=CH),
                        in0=hs4,
                        in1=wv[:]
                        .rearrange("p (t h) -> p t h", h=HEADS)[:, :, :, None]
                        .to_broadcast([P, T, HEADS, CH]),
                        op=OP.mult,
                    )
                    ag = p2ps.tile([P, HC], f32, tag="ag")
                    for t in range(T):
                        nc.tensor.matmul(
                            ag[:],
                            mm[:, t * P : (t + 1) * P],
                            ms[:, t * HC : (t + 1) * HC],
                            start=(t == 0),
                            stop=(t == T - 1),
                        )
                    ag3 = ag[:].rearrange("p (h c) -> p h c", c=CH)
                    dpre = p2.tile([P, HEADS], f32, tag="dpre")
                    nc.vector.tensor_scalar_add(
                        dpre[:].rearrange("p (h o) -> p h o", o=1),
                        ag3[:, :, HID : HID + 1],
                        EPS,
                    )
                    dr = p2.tile([P, HEADS], f32, tag="dr")
                    nc.vector.reciprocal(dr[:], dpre[:])
                    h1 = p2.tile([P, HD], f32, tag="h1")
                    nc.vector.tensor_tensor(
                        out=h1[:].rearrange("p (h c) -> p h c", c=HID),
                        in0=ag3[:, :, 0:HID],
                        in1=dr[:, :, None].to_broadcast([P, HEADS, HID]),
                        op=OP.mult,
                    )
                    nc.vector.tensor_add(h1[:], h1[:], b1t[:])
                    # ELU(x) = max(x,0) + exp(min(x,0)) - 1
                    tmn = p2.tile([P, HD], f32, tag="tmn")
                    nc.vector.tensor_scalar_min(tmn[:], h1[:], 0.0)
                    tex = p2.tile([P, HD], f32, tag="tex")
                    nc.scalar.activation(tex[:], tmn[:], AF.Exp)
                    trl = p2.tile([P, HD], f32, tag="trl")
                    nc.vector.tensor_scalar_max(trl[:], h1[:], 0.0)
                    nc.vector.tensor_add(tex[:], tex[:], trl[:])
                    nc.vector.tensor_scalar_add(tex[:], tex[:], -1.0)
                    for half in range(2):
                        tp = p2ps.tile([P, P], f32, tag="tp")
                        nc.tensor.transpose(
                            tp[:], tex[:, half * P : (half + 1) * P], idt[:]
                        )
                        ccp = p2.tile([P, P], bf16, tag="ccp")
                        nc.vector.tensor_copy(ccp[:], tp[:])
                        nc.sync.dma_start(h1T[half * P : (half + 1) * P, nsl], ccp[:])
            tc.strict_bb_all_engine_barrier()

            # ---- Phase 3: h2_cat = h1 @ W2_cat for this core's nodes
            with tc.tile_pool(name="p3", bufs=3) as p3, tc.tile_pool(
                name="p3ps", bufs=2, space="PSUM"
            ) as p3ps:
                for ntl in range(npc // P):
                    sl = slice(ntl * P, (ntl + 1) * P)
                    ha = p3.tile([P, P], bf16, tag="ha")
                    nc.scalar.dma_start(ha[:], h1T[0:P, sl])
                    hb2 = p3.tile([P, P], bf16, tag="hb2")
                    nc.scalar.dma_start(hb2[:], h1T[P : 2 * P, sl])
                    ps3 = p3ps.tile([P, C2], f32, tag="p3ps")
                    nc.tensor.matmul(ps3[:], ha[:], w2a[:], start=True, stop=False)
                    nc.tensor.matmul(ps3[:], hb2[:], w2b[:], start=False, stop=True)
                    hc2 = p3.tile([P, C2], bf16, tag="hc2")
                    nc.vector.tensor_copy(hc2[:], ps3[:])
                    nc.sync.dma_start(h2loc[sl, :], hc2[:])
            tc.strict_bb_all_engine_barrier()

            # ---- Phase 4: AllGather the per-core h2_cat slices
            nc.gpsimd.collective_compute(
                "AllGather",
                OP.bypass,
                replica_groups=[list(range(NCORES))],
                ins=[h2loc[:]],
                outs=[h2tab[:]],
            )
            tc.strict_bb_all_engine_barrier()

            # ---- Phase 5: layer-2 edge aggregation; ln() deferred
            with tc.tile_pool(name="p5", bufs=2) as p5, tc.tile_pool(
                name="p5ps", bufs=2, space="PSUM"
            ) as p5ps:
                for b in range(B):
                    mt = p5.tile([P, 3 * T], i32, tag="mt5")
                    nc.scalar.dma_start(mt[:], meta[b])
                    it = mt[:, 0:T]
                    dg = mt[:, T : 2 * T]
                    dl = mt[:, 2 * T : 3 * T].bitcast(f32)
                    dlb = p5.tile([P, T], bf16, tag="dlb5")
                    nc.vector.tensor_copy(dlb[:], dl)
                    hs = p5.tile([P, T * C2], bf16, tag="hs5")
                    nc.gpsimd.indirect_dma_start(
                        out=hs[:],
                        out_offset=None,
                        in_=h2tab[:, :],
                        in_offset=bass.IndirectOffsetOnAxis(ap=it, axis=0),
                    )
                    ad2 = p5.tile([P, T], bf16, tag="ad5")
                    nc.gpsimd.indirect_dma_start(
                        out=ad2[:],
                        out_offset=None,
                        in_=h2tab[:, :],
                        in_offset=bass.IndirectOffsetOnAxis(ap=dg, axis=0),
                        element_offset=NCLASS + 1,
                    )
                    hs3 = hs[:].rearrange("p (t c) -> p t c", c=C2)
                    lg = p5.tile([P, T], f32, tag="lg5")
                    nc.vector.tensor_tensor(
                        out=lg[:].rearrange("p (t o) -> p t o", o=1),
                        in0=hs3[:, :, NCLASS : NCLASS + 1],
                        in1=ad2[:, :, None],
                        op=OP.add,
                    )
                    lr = p5.tile([P, T], f32, tag="lr5")
                    nc.vector.tensor_scalar_mul(lr[:], lg[:], NEG_SLOPE)
                    nc.vector.tensor_tensor(out=lr[:], in0=lr[:], in1=lg[:], op=OP.max)
                    wv = p5.tile([P, T], bf16, tag="wv5")
                    nc.scalar.activation(wv[:], lr[:], AF.Exp)
                    nc.vector.memset(hs3[:, :, NCLASS : NCLASS + 1], 1.0)
                    mm = p5.tile([P, T * P], bf16, tag="mm5")
                    nc.vector.tensor_tensor(
                        out=mm[:].rearrange("p (t j) -> p t j", j=P),
                        in0=dlb[:, :, None].to_broadcast([P, T, P]),
                        in1=ct[:, None, :].to_broadcast([P, T, P]),
                        op=OP.is_equal,
                    )
                    ms = p5.tile([P, T * C2], bf16, tag="ms5")
                    nc.vector.tensor_tensor(
                        out=ms[:].rearrange("p (t c) -> p t c", c=C2),
                        in0=hs3,
                        in1=wv[:, :, None].to_broadcast([P, T, C2]),
                        op=OP.mult,
                    )
                    ag = p5ps.tile([P, C2], f32, tag="ag5")
                    for t in range(T):
                        nc.tensor.matmul(
                            ag[:],
                            mm[:, t * P : (t + 1) * P],
                            ms[:, t * C2 : (t + 1) * C2],
                            start=(t == 0),
                            stop=(t == T - 1),
                        )
                    dpre = p5.tile([P, 1], f32, tag="dpre5")
                    nc.vector.tensor_scalar_add(
                        dpre[:], ag[:, NCLASS : NCLASS + 1], EPS
                    )
                    dr = p5.tile([P, 1], f32, tag="dr5")
                    nc.vector.reciprocal(dr[:], dpre[:])
                    o2 = o2all[:, b * NCLASS : (b + 1) * NCLASS]
                    nc.vector.tensor_tensor(
                        out=o2,
                        in0=ag[:, 0:NCLASS],
                        in1=dr[:].to_broadcast([P, NCLASS]),
                        op=OP.mult,
                    )
                    nc.vector.tensor_add(o2, o2, b2t[:])
                    mx = p5.tile([P, 1], f32, tag="mx")
                    nc.vector.tensor_reduce(mx[:], o2, axis=X, op=OP.max)
                    nc.vector.tensor_scalar_mul(nmx[:, b : b + 1], mx[:], -1.0)
                    exs = p5.tile([P, NCLASS], f32, tag="exs")
                    nc.scalar.activation(
                        exs[:],
                        o2,
                        AF.Exp,
                        bias=nmx[:, b : b + 1],
                        accum_out=smx[:, b : b + 1],
                    )
                # batched ln over all blocks, then per-block shift + store
                lse = cp.tile([P, B], f32)
                nc.scalar.activation(lse[:], smx[:], AF.Ln)
                sh = cp.tile([P, B], f32)
                nc.vector.tensor_tensor(
                    out=sh[:], in0=nmx[:], in1=lse[:], op=OP.subtract
                )
                for b in range(B):
                    fo = p5.tile([P, NCLASS], f32, tag="fo")
                    nc.vector.tensor_scalar_add(
                        fo[:], o2all[:, b * NCLASS : (b + 1) * NCLASS],
                        sh[:, b : b + 1],
                    )
                    nc.sync.dma_start(outx[b * P : (b + 1) * P, :], fo[:])
    nc.compile()
    return nc


def _host_prep(inputs):
    from ml_dtypes import bfloat16

    x = np.ascontiguousarray(np.asarray(inputs["x"], np.float32))
    ei = np.asarray(inputs["edge_index"]).astype(np.int64)
    W1 = np.asarray(inputs["W1"], np.float32)
    as1 = np.asarray(inputs["att_src1"], np.float32)
    ad1 = np.asarray(inputs["att_dst1"], np.float32)
    b1 = np.asarray(inputs["b1"], np.float32)
    W2 = np.asarray(inputs["W2"], np.float32)
    as2 = np.asarray(inputs["att_src2"], np.float32).reshape(-1)
    ad2 = np.asarray(inputs["att_dst2"], np.float32).reshape(-1)
    b2 = np.asarray(inputs["b2"], np.float32)

    n = x.shape[0]
    B = -(-n // (NCORES * P))  # blocks per core
    npc = B * P
    npad = NCORES * npc

    loops = np.arange(n, dtype=np.int64)
    src = np.concatenate([ei[0], loops])
    dst = np.concatenate([ei[1], loops])
    gidx, dstg, dstl, T = _prep_graph(src, dst, NCORES * B)
    meta = np.concatenate(
        [gidx, dstg, dstl.view(np.int32)], axis=2
    )  # [nblocks, P, 3T]

    w1cat = np.zeros((IN_F, HCX), np.float32)
    for h in range(HEADS):
        w1cat[:, h * CH : h * CH + HID] = W1[:, h * HID : (h + 1) * HID]
        w1cat[:, h * CH + HID] = W1[:, h * HID : (h + 1) * HID] @ as1[h]
        w1cat[:, HC + h] = W1[:, h * HID : (h + 1) * HID] @ ad1[h]
    w2cat = np.zeros((HD, C2), np.float32)
    w2cat[:, :NCLASS] = W2
    w2cat[:, NCLASS] = W2 @ as2
    w2cat[:, NCLASS + 1] = W2 @ ad2

    xT = np.zeros((IN_F, npad), bfloat16)
    xT[:, :n] = x.T.astype(bfloat16)

    host = dict(
        xT=xT,
        w1c=w1cat.astype(bfloat16),
        w2c=w2cat.astype(bfloat16),
        b1r=np.tile(b1[None, :], (P, 1)).astype(np.float32),
        b2r=np.tile(b2[None, :], (P, 1)).astype(np.float32),
        c128=np.tile(np.arange(P, dtype=np.float32)[None, :], (P, 1)).astype(
            bfloat16
        ),
        idn=np.eye(P, dtype=np.float32),
    )
    in_maps = []
    for c in range(NCORES):
        m = dict(host)
        m["meta"] = np.ascontiguousarray(meta[c * B : (c + 1) * B])
        in_maps.append(m)
    return in_maps, n, npad, npc, B, T


def _forward(inputs, trace=False):
    from concourse.bass_utils import run_bass_kernel_spmd

    in_maps, n, npad, npc, B, T = _host_prep(inputs)
    nc = _build_program(npad, npc, B, T)
    res = run_bass_kernel_spmd(nc, in_maps, list(range(NCORES)), trace=trace)
    out = np.concatenate([res.results[c]["out"] for c in range(NCORES)], axis=0)
    return out[:n].astype(np.float32), res


def kernel(**inputs) -> np.ndarray:
    out, _ = _forward(inputs, trace=False)
    return out
